# revision 27
# baseline (speedup 1.0000x reference)
"""Multi-head attention (N=2, K=2048, M=1024, H=16, D=64) on 8 TRN2 cores.

Sharding: tensor-parallel over heads — core c owns heads (2c, 2c+1).
Each core computes q/k/v projections for its 2 heads (full sequence),
attention, and a rank-128 partial of the output projection (its 128 rows
of Wo's input dim). Host sums the 8 partials and adds bo. No device
collectives.

On-device layouts (per core):
  xq/xk/xv [8 tb, 128 p, 8 mc, 512 f] bf16  host-tiled transposed inputs:
           [tb, p, mc, f] = x[tok=tb*512+f, m=mc*128+p], tok = n*2048+k
           -> one contiguous 1MB DMA per (tensor, tb)
  wq/wk/wv [1024 m, 128 hd] bf16   W[h,d,m] -> [m, hl*64+d] for local heads
  wo       [128 hd, 1024 mo] bf16  Wo[:, c*128:(c+1)*128].T
  bqk      [128, 2] f32, bv2 [64, 2] f32
  outT     [8 tb, 128 p, 8 mb, 512 f] bf16  tiled partial (1MB DMA per tb)

Attention is ONE continuous software pipeline over 128 iterations
(2 batches x 4 kq-quarters x 16 l-blocks) with no barriers: per
iteration the two heads' K=64 score matmuls run CONCURRENTLY on
disjoint PE row-groups (base partitions 0/64 -> row packing), one
[128,1024] ACTIVATE exps both heads, and AV matmuls (ones-column
trick, M=65) lag 12 iterations behind. Quarter normalization
(reciprocal_approx_fast + gpsimd partition_broadcast + PSUM-direct
multiply) and output-projection stripes are emitted mid-stream.
Projections are single-matmul fill steps paced into the PE's slack,
INTERLEAVED between scores/AVs inside each iteration so mm-PSUM
eviction WARs are satisfied by arrival; a deadline `require()` forces
emission of any unit a consumer needs.

Trace-driven changes vs the 242us baseline (median ~233us, best 229us;
run-to-run spread is dominated by hardware power-cap windows that slow
the core to 50% for ~20-35us, usually over the tail):
 - startup: DMA issues cost ~0.6us each on the issuing engine; split
   them across the two HW-DGE engines (SP=sync, Activation=scalar),
   tiny bqk first (it gates the first projection eviction), 256KB x
   chunks (the DMA subsystem ramps slowly in the first ~15us).  PE
   p-state is ramped with a few junk warm-up matmuls off an on-chip
   identity while the first x chunks are in flight (a cold PE runs at
   0.65-1.2GHz for several us).
 - V transposes pair both heads into [128,128] PE transposes (32 of
   them instead of 64 [64,128] ones): ~4x fewer PE transpose cycles.
 - wq/wk/wv arrive host-pre-tiled [128 m-part, 8 mc, 128 hd] so their
   DMAs are contiguous (the strided rearrange took ~3.5us and gated
   the first projection).
 - fill steps are interleaved between the stream's matmuls so their
   PSUM-eviction WARs are satisfied by arrival.
 - tail: final-quarter norm reads yacc PSUM directly (no release copy
   needed - nothing follows) with the second denominator copy on the
   idle ScalarE, and the last out-proj stripe gets 4 PSUM slots
   (reusing the dead score banks) with ACT/DVE alternating evictions
   and a 4-chunk output DMA.
Output partials are cast to bf16 (halves the out-DMA); the host sums
partials in f32.
"""
from collections import deque

import numpy as np
import ml_dtypes

import concourse.bass as bass
import concourse.tile as tile
from concourse.masks import make_identity
from concourse import bacc, mybir
from concourse.bass_utils import run_bass_kernel_spmd

F32 = mybir.dt.float32
BF16 = mybir.dt.bfloat16
BFNP = ml_dtypes.bfloat16

N_CORES = 8
DM = 1024          # d_model
TOK = 4096         # N*K tokens
SEQ = 2048         # tokens per batch
NB = 2             # batches
HC = 2             # heads per core
D = 64             # head dim

AV_LAG = 12        # iterations AV trails scores/exp
AV_LAG_TAIL = 2    # shrink lag near the end to cut the drain tail
N_WARMUP = 4       # junk matmuls to ramp the PE p-state during DMA wait

_prog_cache = {}


class FillSched:
    """Named-unit fill scheduler. Units are atomic (they share the mm_ps
    pool and must not interleave with each other); steps within the
    active unit are paced out by PE cost (matmul steps cost 1, DVE/DMA
    steps cost 0). require(name) forces full emission of every unit up
    to and including `name` — emission order defines Tile dependencies,
    so any unit a consumer reads from MUST be emitted (not just queued)
    before the consumer."""

    def __init__(self):
        self.order = deque()      # (name, deque((fn, cost)))
        self.cur_name = None
        self.cur = deque()
        self.done = set()

    def add(self, name, unit):
        self.order.append((name, deque(unit)))

    def add_front(self, name, unit):
        self.order.appendleft((name, deque(unit)))

    def _finish_cur(self):
        while self.cur:
            self.cur.popleft()[0]()
        if self.cur_name is not None:
            self.done.add(self.cur_name)
            self.cur_name = None

    def pop_steps(self, budget):
        while budget > 0:
            if not self.cur:
                if self.cur_name is not None:
                    self.done.add(self.cur_name)
                    self.cur_name = None
                if not self.order:
                    return
                self.cur_name, self.cur = self.order.popleft()
            fn, cost = self.cur.popleft()
            fn()
            budget -= cost
        if not self.cur and self.cur_name is not None:
            self.done.add(self.cur_name)
            self.cur_name = None

    def require(self, name):
        if name in self.done:
            return
        if self.cur_name is not None:
            if self.cur_name == name:
                self._finish_cur()
                return
            self._finish_cur()
        while name not in self.done:
            assert self.order, f"unit {name} was never queued"
            self.cur_name, self.cur = self.order.popleft()
            self._finish_cur()

    def drain(self):
        self._finish_cur()
        while self.order:
            self.cur_name, self.cur = self.order.popleft()
            self._finish_cur()


def build_program():
    nc = bacc.Bacc("TRN2", target_bir_lowering=False, debug=False)

    xq = nc.dram_tensor("xq", [8, 128, 8, 512], BF16, kind="ExternalInput")
    xk = nc.dram_tensor("xk", [8, 128, 8, 512], BF16, kind="ExternalInput")
    xv = nc.dram_tensor("xv", [8, 128, 8, 512], BF16, kind="ExternalInput")
    # pre-tiled on host: w[p, c, d] = W[c*128+p, d] -> contiguous DMA
    wq = nc.dram_tensor("wq", [128, 8, 128], BF16, kind="ExternalInput")
    wk = nc.dram_tensor("wk", [128, 8, 128], BF16, kind="ExternalInput")
    wv = nc.dram_tensor("wv", [128, 8, 128], BF16, kind="ExternalInput")
    wo = nc.dram_tensor("wo", [128, DM], BF16, kind="ExternalInput")
    bqk = nc.dram_tensor("bqk", [128, 2], F32, kind="ExternalInput")
    bv = nc.dram_tensor("bv", [128, 1], F32, kind="ExternalInput")
    outT = nc.dram_tensor("outT", [8, 128, 8, 512], BF16, kind="ExternalOutput")

    Exp = mybir.ActivationFunctionType.Exp

    with tile.TileContext(nc) as tc:
        with (
            tc.tile_pool(name="const", bufs=1) as const,
            tc.tile_pool(name="big", bufs=1) as big,
            tc.tile_pool(name="xpool", bufs=1) as xpool,
            tc.tile_pool(name="attn", bufs=AV_LAG + 2) as attnp,
            tc.tile_pool(name="norm", bufs=2) as normp,
            tc.tile_pool(name="osb", bufs=2) as osb,
            tc.tile_pool(name="vtpool", bufs=3) as vtpool,
            tc.tile_pool(name="mm_ps", bufs=2, space="PSUM") as mm_ps,
            tc.tile_pool(name="sc_ps", bufs=2, space="PSUM") as sc_ps,
            tc.tile_pool(name="y_ps", bufs=2, space="PSUM") as y_ps,
        ):
            # ---- weights / biases.  Issue order per engine == execution
            # order; sync and scalar each issue a DMA in ~0.6us, transfers
            # are FIFO per queue with ~5us first-transfer latency, and the
            # DGE queue blocks after ~4 outstanding.  So: tiny bqk first
            # (it gates the first projection eviction), 2 big chunks per
            # x tile, k-side on sync / q-side on scalar. ----
            bqk_sb = const.tile([128, 2], F32, tag="bqk")
            nc.scalar.dma_start(bqk_sb[:], bqk[:, :])
            wk_sb = const.tile([128, 8, 128], BF16, tag="wk")
            nc.sync.dma_start(wk_sb[:], wk[:])
            wq_sb = const.tile([128, 8, 128], BF16, tag="wq")
            nc.scalar.dma_start(wq_sb[:], wq[:])

            # ---- persistent activations ----
            qT = big.tile([128, TOK], BF16, tag="qT")     # [hd, tok]
            kT = big.tile([128, TOK], BF16, tag="kT")     # [hd, tok]
            # v blocks: 32 token-blocks of [128 tok, 2*(64+1)]; col 64 of each
            # per-head group is the ones column (softmax denominator trick)
            vA = big.tile([128, 32 * 130], BF16, tag="vA")
            yT = big.tile([128, TOK], BF16, tag="yT")     # attn out [hd, tok]

            nc.vector.memset(
                vA[:].rearrange("p (b h c) -> p b h c", h=2, c=65)[:, :, :, 64:65], 1.0
            )

            prefetched = {}

            # chunked first tiles: 4 chunks of 256KB so projections can
            # start as soon as the first chunk lands (the DMA subsystem
            # ramps slowly in the first ~15us — small chunks flow better)
            def prefetch_chunked(key, tb, eng):
                dram = {"q": xq, "k": xk, "v": xv}[key]
                t = xpool.tile([128, 8, 512], BF16,
                               tag=f"x{key}{tb % 4}", name="xt")
                for j in range(4):
                    eng.dma_start(t[:, 2 * j:2 * j + 2, :],
                                  dram[tb][:, 2 * j:2 * j + 2, :])
                prefetched[(key, tb)] = t

            prefetch_chunked("k", 0, nc.sync)
            prefetch_chunked("q", 0, nc.scalar)

            # [128,128] identity for the paired-head V transposes (built
            # on gpsimd - no DMA), and a junk warm-up operand (memset on
            # the idle DVE) so warm-ups need no DMA at all
            ident = const.tile([128, 128], BF16, tag="ident")
            make_identity(nc, ident[:])
            warm_sb = const.tile([128, 512], BF16, tag="warm")
            nc.vector.memset(warm_sb[:], 0.25)

            # ---- PE p-state warm-up: junk matmuls while the first x
            # chunks are in flight (never read; sc tiles are recycled).
            # A cold PE runs at 0.65-1.2GHz until ~3us of continuous work.
            def warm_mms(k):
                for w in range(k):
                    wps = sc_ps.tile([128, 1024], F32, tag="sc", name="warm")
                    nc.tensor.matmul(wps[:, 0:512], lhsT=ident[:],
                                     rhs=warm_sb[:], start=True, stop=True)
                    nc.tensor.matmul(wps[:, 512:1024], lhsT=ident[:],
                                     rhs=warm_sb[:], start=True, stop=True)

            warm_mms(N_WARMUP)

            def prefetch(key, tb):
                dram = {"q": xq, "k": xk, "v": xv}[key]
                # per-slot tag (bufs=1): batch-1's (key, tb+4) reuses exactly
                # the buffer of (key, tb), with a WAR dep on its reads
                t = xpool.tile([128, 8, 512], BF16,
                               tag=f"x{key}{tb % 4}", name="xt")
                nc.sync.dma_start(t[:], dram[tb])
                prefetched[(key, tb)] = t

            # remaining batch-0 prefetches + late weights, all on sync, in
            # deadline order (k1 @it4, k2 @it8, v0 @it12(AV), k3 @it12,
            # q1 @it16, wo @ first out-proj ~it30, ...)
            prefetch("k", 1)
            prefetch("k", 2)
            wv_sb = const.tile([128, 8, 128], BF16, tag="wv")
            nc.sync.dma_start(wv_sb[:], wv[:])
            prefetch("v", 0)
            bv_sb = const.tile([128, 1], F32, tag="bv")
            nc.sync.dma_start(bv_sb[:], bv[:, :])
            prefetch("k", 3)
            prefetch("v", 1)
            prefetch("q", 1)
            prefetch("v", 2)
            prefetch("q", 2)
            prefetch("v", 3)
            prefetch("q", 3)
            wo_sb = const.tile([128, DM], BF16, tag="wo")
            nc.sync.dma_start(wo_sb[:], wo[:, :])

            def proj_qk_steps(tb, which):
                """One qk projection as 8 single-MM closures (last one evicts)."""
                key, w_sb, dstT, bcol = (
                    ("q", wq_sb, qT, 0),
                    ("k", wk_sb, kT, 1),
                )[which]
                state = {}

                def step(mc):
                    if mc == 0:
                        state["xt"] = prefetched.pop((key, tb))
                        state["ps"] = mm_ps.tile([128, 512], F32, tag="mm", name="ps")
                    nc.tensor.matmul(
                        state["ps"][:], lhsT=w_sb[:, mc, :],
                        rhs=state["xt"][:, mc, :],
                        start=(mc == 0), stop=(mc == 7),
                    )
                    if mc == 7:
                        nc.vector.tensor_scalar_add(
                            dstT[:, tb * 512:(tb + 1) * 512], state["ps"][:],
                            bqk_sb[:, bcol:bcol + 1],
                        )
                        if tb < 4:
                            # batch-1 prefetch reuses this tile's buffer; it
                            # must be EMITTED after the last read of the old
                            # tile or the DMA races the projection
                            prefetch(key, tb + 4)
                return [(lambda mc=mc: step(mc), 1) for mc in range(8)]

            vstate = {}

            def proj_v_mm_steps(tb):
                """V projection matmuls: 8 single-MM closures + a bias
                evict to an SBUF staging tile (DVE)."""
                state = vstate.setdefault(tb, {})

                def step(mc):
                    if mc == 0:
                        state["xt"] = prefetched.pop(("v", tb))
                        state["ps"] = mm_ps.tile([128, 512], F32, tag="mm", name="ps")
                    nc.tensor.matmul(
                        state["ps"][:], lhsT=wv_sb[:, mc, :],
                        rhs=state["xt"][:, mc, :],
                        start=(mc == 0), stop=(mc == 7),
                    )
                    if mc == 7:
                        state["vt"] = vtpool.tile(
                            [128, 512], BF16, tag="vt", name="vt")
                        nc.vector.tensor_scalar_add(
                            state["vt"][:], state["ps"][:], bv_sb[:, 0:1])
                    if mc == 7 and tb < 4:
                        prefetch("v", tb + 4)

                return [(lambda mc=mc: step(mc), 1) for mc in range(8)]

            def proj_v_t_steps(tb):
                """V transposes: 4 paired-head [128,128] PE transposes + 2
                DVE scatter copies each into the 65-stride vA layout.
                A SEPARATE fill unit scheduled one unit after the matmuls,
                so the in-order PE doesn't reach the first transpose before
                the DVE bias-evict has written the vt staging tile."""
                state = vstate  # read via tb at call time

                def tstep(j):
                    base = (tb * 4 + j) * 130
                    vt = vstate[tb]["vt"]
                    # [128 hd, 128 tok] -> [128 tok, 128 hd] in one shot
                    tp = mm_ps.tile([128, 128], BF16, tag="mm", name="tp")
                    nc.tensor.transpose(
                        tp[:], vt[:, j * 128:(j + 1) * 128], ident[:])
                    for hl in range(2):
                        nc.vector.tensor_copy(
                            vA[:, base + hl * 65: base + hl * 65 + 64],
                            tp[:, hl * 64:(hl + 1) * 64])

                return [(lambda j=j: tstep(j), 1) for j in range(4)]

            def out_proj_steps(n, qtr, tail=False):
                """One 512-token output-projection stripe: 8 (MM + bf16 copy)
                closures; the last also DMAs the stripe out. In tail mode the
                exp stream is over, so the dead score PSUM banks give 4 mm
                slots, ScalarE helps with the casts, and the DMA is split
                into 4 chunks to overlap them."""
                tb = n * 4 + qtr
                state = {}

                def step(mb):
                    if mb == 0:
                        state["o"] = osb.tile(
                            [128, 8, 512], BF16, tag="o", name="o_sb")
                    if tail:
                        if mb % 2 == 0:
                            state["ps"] = sc_ps.tile(
                                [128, 1024], F32, tag="sc", name="ps")
                        ps = state["ps"][:, (mb % 2) * 512:(mb % 2) * 512 + 512]
                    else:
                        ps = mm_ps.tile([128, 512], F32, tag="mm", name="ps")[:]
                    nc.tensor.matmul(
                        ps, lhsT=wo_sb[:, mb * 128:(mb + 1) * 128],
                        rhs=yT[:, tb * 512:(tb + 1) * 512],
                        start=True, stop=True,
                    )
                    if tail and mb % 2 == 0:
                        nc.scalar.copy(state["o"][:, mb, :], ps)
                    else:
                        nc.vector.tensor_copy(state["o"][:, mb, :], ps)
                    if tail and mb % 2 == 1:
                        nc.sync.dma_start(
                            outT[tb, :, mb - 1:mb + 1, :],
                            state["o"][:, mb - 1:mb + 1, :])
                    elif not tail and mb == 7:
                        nc.sync.dma_start(outT[tb], state["o"][:])
                return [(lambda mb=mb: step(mb), 1) for mb in range(8)]

            fill = FillSched()
            pending_fronts = []

            def norm_qtr(n, qtr, yaccs):
                kq0 = n * SEQ + qtr * 512
                tail = (n == NB - 1 and qtr == 3)
                if not tail:
                    ycps = []
                    for h in range(2):
                        # both release copies FIRST: they free the yacc PSUM
                        # buffers the next quarter's first AVs are waiting on;
                        # the rest of the chain runs off the PE critical path.
                        # (gpsimd can't access PSUM; custom DVE ops can't
                        # either)
                        ycp = normp.tile([65, 512], F32, tag="ycp", name="ycp")
                        nc.vector.tensor_copy(ycp[:], yaccs[h][:])
                        ycps.append(ycp)
                else:
                    # final quarter: nothing follows — read yacc PSUM
                    # directly, skip the release copies, and pull the
                    # second denominator copy onto the idle ScalarE.
                    # (The tail runs under a ~50% hardware power cap, so
                    # keep-warm junk matmuls don't help — measured.)
                    ycps = yaccs
                dsbs = []
                for h in range(2):
                    # custom DVE op needs a partition-0 input
                    dsb = normp.tile([1, 512], F32, tag="dsb", name="dsb")
                    if tail and h == 1:
                        nc.scalar.copy(dsb[:], ycps[h][64:65, :])
                    else:
                        nc.vector.tensor_copy(dsb[:], ycps[h][64:65, :])
                    dsbs.append(dsb)
                for h in range(2):
                    hp = h * 64
                    ycp = ycps[h]
                    recip = normp.tile([1, 512], F32, tag="recip", name="recip")
                    nc.vector.reciprocal_approx_fast(recip[:], dsbs[h])
                    bcast = normp.tile([64, 512], F32, tag="bcast", name="bcast")
                    nc.gpsimd.partition_broadcast(bcast[:], recip[:])
                    nc.vector.tensor_mul(
                        yT[hp:hp + 64, kq0:kq0 + 512],
                        ycp[0:64, :], bcast[:],
                    )
                if tail:
                    fill.add_front(f"out{n}{qtr}", out_proj_steps(n, qtr, tail))
                else:
                    # defer one iteration: the in-order PE must not reach
                    # the stripe's first matmul before the ~4us DVE/gpsimd
                    # norm chain has written yT (measured ~1.3us stalls at
                    # every quarter boundary with immediate add_front)
                    pending_fronts.append(
                        (f"out{n}{qtr}", out_proj_steps(n, qtr, tail)))

            def do_av(at, n, qtr, lb, yaccs):
                lt = n * 16 + lb
                fill.require(f"v{n * 4 + lb // 4}")
                for h in range(2):
                    nc.tensor.matmul(
                        yaccs[h][:],
                        lhsT=vA[:, lt * 130 + h * 65: lt * 130 + h * 65 + 65],
                        rhs=at[:, h * 512:(h + 1) * 512],
                        start=(lb == 0), stop=(lb == 15),
                    )
                if lb == 15:
                    norm_qtr(n, qtr, yaccs)

            # ---- upfront: k0 + q0 projections only ----
            for f, _ in proj_qk_steps(0, 1):
                f()
            for f, _ in proj_qk_steps(0, 0):
                f()
            fill.done.update({"k0", "q0"})

            # deadline order: scores(lb) need k-tb(lb//4) / q-tb(qtr);
            # AV (lag 12) needs v-tb((lb-12)//4); batch 1 follows
            # (batch-1 prefetches are emitted by the consumption hooks
            # inside proj_*_steps — buffer-exact, race-free)
            units = [
                ("k1", proj_qk_steps(1, 1)), ("k2", proj_qk_steps(2, 1)),
                ("v0m", proj_v_mm_steps(0)), ("k3", proj_qk_steps(3, 1)),
                ("v0", proj_v_t_steps(0)),
                ("v1m", proj_v_mm_steps(1)), ("q1", proj_qk_steps(1, 0)),
                ("v1", proj_v_t_steps(1)),
                ("v2m", proj_v_mm_steps(2)), ("q2", proj_qk_steps(2, 0)),
                ("v2", proj_v_t_steps(2)),
                ("v3m", proj_v_mm_steps(3)), ("q3", proj_qk_steps(3, 0)),
                ("v3", proj_v_t_steps(3)),
            ]
            units += [
                ("k4", proj_qk_steps(4, 1)), ("q4", proj_qk_steps(4, 0)),
                ("k5", proj_qk_steps(5, 1)), ("k6", proj_qk_steps(6, 1)),
                ("v4m", proj_v_mm_steps(4)), ("k7", proj_qk_steps(7, 1)),
                ("v4", proj_v_t_steps(4)),
                ("v5m", proj_v_mm_steps(5)), ("q5", proj_qk_steps(5, 0)),
                ("v5", proj_v_t_steps(5)),
                ("v6m", proj_v_mm_steps(6)), ("q6", proj_qk_steps(6, 0)),
                ("v6", proj_v_t_steps(6)),
                ("v7m", proj_v_mm_steps(7)), ("q7", proj_qk_steps(7, 0)),
                ("v7", proj_v_t_steps(7)),
            ]
            for name, unit in units:
                fill.add(name, unit)

            # ---- the continuous attention stream ----
            pend = deque()
            qtr_yaccs = {}
            NIT = NB * 64
            for i in range(NIT):
                while pending_fronts:
                    fill.add_front(*pending_fronts.pop())
                n, r = divmod(i, 64)
                qtr, lb = divmod(r, 16)
                lt = n * 16 + lb
                kq0 = n * SEQ + qtr * 512
                if lb == 0:
                    qtr_yaccs[(n, qtr)] = [
                        y_ps.tile([65, 512], F32, tag="yacc", name="yacc")
                        for _ in range(2)
                    ]
                fill.require(f"k{n * 4 + lb // 4}")
                fill.require(f"q{n * 4 + qtr}")
                budget = 4 if i < 24 else (3 if i < 48 else 2)
                # interleave fill steps between the stream's matmuls so
                # their PSUM-eviction WARs are satisfied by arrival
                fill.pop_steps(1)
                budget -= 1
                sp = sc_ps.tile([128, 1024], F32, tag="sc", name="sp")
                for h in range(2):
                    # K=64, base partitions 0/64 -> concurrent row-tiles
                    nc.tensor.matmul(
                        sp[:, h * 512:(h + 1) * 512],
                        lhsT=kT[h * 64:(h + 1) * 64, lt * 128:(lt + 1) * 128],
                        rhs=qT[h * 64:(h + 1) * 64, kq0:kq0 + 512],
                        start=True, stop=True,
                    )
                at = attnp.tile([128, 1024], BF16, tag="at", name="at")
                nc.scalar.activation(at[:], sp[:], Exp, scale=0.125)
                pend.append((at, n, qtr, lb, qtr_yaccs[(n, qtr)]))
                fill.pop_steps(1)
                budget -= 1
                limit = AV_LAG if i < NIT - (AV_LAG - AV_LAG_TAIL) else AV_LAG_TAIL
                # drain AVs two l-blocks at a time (every other iteration):
                # grouping full-array AV matmuls halves the PE row-config
                # switches against the half-array score pairs
                if i % 2 == 1 or len(pend) > limit + 1:
                    while len(pend) > limit - 1 and pend:
                        args = pend.popleft()
                        do_av(*args)
                        if args[3] == 15:
                            del qtr_yaccs[(args[1], args[2])]
                if budget:
                    fill.pop_steps(budget)
            for args in pend:
                do_av(*args)
            while pending_fronts:
                fill.add_front(*pending_fronts.pop())
            fill.drain()

    nc.compile()
    return nc


def get_program():
    if "nc" not in _prog_cache:
        _prog_cache["nc"] = build_program()
    return _prog_cache["nc"]


def _tile_x(x):
    # [TOK, DM] f32 -> bf16 tiles [8 tb, 128 p, 8 mc, 512 f]:
    # t[tb,p,mc,f] = x[tb*512+f, mc*128+p]
    t = x.reshape(8, 512, 8, 128).astype(BFNP)   # [tb, f, mc, p]
    return np.ascontiguousarray(np.transpose(t, (0, 3, 2, 1)))


def make_in_maps(query, key, value, Wq, bq, Wk, bk, Wv, bv, Wo):
    """Host-side shard + layout. Returns list of 8 per-core input dicts."""
    xq = _tile_x(query.reshape(TOK, DM))
    xk = _tile_x(key.reshape(TOK, DM))
    xv = _tile_x(value.reshape(TOK, DM))

    def _tile_w(W, h0):
        # W[h,d,m] slice -> [m, hl*64+d] -> pre-tiled [p, mc, hd]
        wf = np.transpose(W[h0:h0 + HC], (2, 0, 1)).reshape(DM, 128)
        return np.ascontiguousarray(
            wf.reshape(8, 128, 128).transpose(1, 0, 2)).astype(BFNP)

    in_maps = []
    for c in range(N_CORES):
        h0 = HC * c
        wq_c = _tile_w(Wq, h0)
        wk_c = _tile_w(Wk, h0)
        wv_c = _tile_w(Wv, h0)
        wo_c = np.ascontiguousarray(
            Wo[:, 128 * c:128 * (c + 1)].T).astype(BFNP)
        bqk_c = np.stack(
            [bq[h0:h0 + HC].reshape(128), bk[h0:h0 + HC].reshape(128)], axis=1
        ).astype(np.float32)
        bv_c = bv[h0:h0 + HC].reshape(128, 1).astype(np.float32)
        in_maps.append({
            "xq": xq, "xk": xk, "xv": xv,
            "wq": wq_c, "wk": wk_c, "wv": wv_c, "wo": wo_c,
            "bqk": bqk_c, "bv": bv_c,
        })
    return in_maps


def untile_out(res_list):
    """Sum per-core tiled bf16 partials in f32 -> [DM, TOK] f32."""
    acc = np.zeros((8, 128, 8, 512), np.float32)
    for r in res_list:
        acc += r["outT"].astype(np.float32)
    # [tb, p, mb, f] -> [mb*128+p, tb*512+f]
    return np.ascontiguousarray(np.transpose(acc, (2, 1, 0, 3))).reshape(DM, TOK)


def kernel(query, key, value, Wq, bq, Wk, bk, Wv, bv, Wo, bo):
    nc = get_program()
    in_maps = make_in_maps(query, key, value, Wq, bq, Wk, bk, Wv, bv, Wo)
    res = run_bass_kernel_spmd(nc, in_maps, list(range(N_CORES)))
    out_t = untile_out(res.results)
    out = out_t.T.reshape(NB, SEQ, DM) + bo.astype(np.float32)
    return out


# revision 28
# speedup vs baseline: 1.1781x; 1.1781x over previous
"""Multi-head attention (N=2, K=2048, M=1024, H=16, D=64) on 8 TRN2 cores.

Sharding: tensor-parallel over heads — core c owns heads (2c, 2c+1).
Each core computes q/k/v projections for its 2 heads (full sequence),
attention, and a rank-128 partial of the output projection (its 128 rows
of Wo's input dim). Host sums the 8 partials and adds bo. No device
collectives.

On-device layouts (per core):
  xq/xk/xv [8 tb, 128 p, 8 mc, 512 f] bf16  host-tiled transposed inputs:
           [tb, p, mc, f] = x[tok=tb*512+f, m=mc*128+p], tok = n*2048+k
           -> one contiguous 1MB DMA per (tensor, tb)
  wq/wk/wv [1024 m, 128 hd] bf16   W[h,d,m] -> [m, hl*64+d] for local heads
  wo       [128 hd, 1024 mo] bf16  Wo[:, c*128:(c+1)*128].T
  bqk      [128, 2] f32, bv2 [64, 2] f32
  outT     [8 tb, 128 p, 8 mb, 512 f] bf16  tiled partial (1MB DMA per tb)

Attention is ONE continuous software pipeline over 128 iterations
(2 batches x 4 kq-quarters x 16 l-blocks) with no barriers: per
iteration the two heads' K=64 score matmuls run CONCURRENTLY on
disjoint PE row-groups (base partitions 0/64 -> row packing), one
[128,1024] ACTIVATE exps both heads, and AV matmuls (ones-column
trick, M=65) lag 12 iterations behind. Quarter normalization
(reciprocal_approx_fast + gpsimd partition_broadcast + PSUM-direct
multiply) and output-projection stripes are emitted mid-stream.
Projections are single-matmul fill steps paced into the PE's slack,
INTERLEAVED between scores/AVs inside each iteration so mm-PSUM
eviction WARs are satisfied by arrival; a deadline `require()` forces
emission of any unit a consumer needs.

Trace-driven changes vs the 242us baseline (median ~233us, best 229us;
run-to-run spread is dominated by hardware power-cap windows that slow
the core to 50% for ~20-35us, usually over the tail):
 - startup: DMA issues cost ~0.6us each on the issuing engine; split
   them across the two HW-DGE engines (SP=sync, Activation=scalar),
   tiny bqk first (it gates the first projection eviction), 256KB x
   chunks (the DMA subsystem ramps slowly in the first ~15us).  PE
   p-state is ramped with a few junk warm-up matmuls off an on-chip
   identity while the first x chunks are in flight (a cold PE runs at
   0.65-1.2GHz for several us).
 - V transposes pair both heads into [128,128] PE transposes (32 of
   them instead of 64 [64,128] ones): ~4x fewer PE transpose cycles.
 - wq/wk/wv arrive host-pre-tiled [128 m-part, 8 mc, 128 hd] so their
   DMAs are contiguous (the strided rearrange took ~3.5us and gated
   the first projection).
 - fill steps are interleaved between the stream's matmuls so their
   PSUM-eviction WARs are satisfied by arrival.
 - tail: final-quarter norm reads yacc PSUM directly (no release copy
   needed - nothing follows) with the second denominator copy on the
   idle ScalarE, and the last out-proj stripe gets 4 PSUM slots
   (reusing the dead score banks) with ACT/DVE alternating evictions
   and a 4-chunk output DMA.
Output partials are cast to bf16 (halves the out-DMA); the host sums
partials in f32.
"""
from collections import deque

import numpy as np
import ml_dtypes

import concourse.bass as bass
import concourse.tile as tile
from concourse.masks import make_identity
from concourse import bacc, mybir
from concourse.bass_utils import run_bass_kernel_spmd

F32 = mybir.dt.float32
BF16 = mybir.dt.bfloat16
BFNP = ml_dtypes.bfloat16

N_CORES = 8
DM = 1024          # d_model
TOK = 4096         # N*K tokens
SEQ = 2048         # tokens per batch
NB = 2             # batches
HC = 2             # heads per core
D = 64             # head dim

AV_LAG = 12        # iterations AV trails scores/exp
AV_LAG_TAIL = 2    # shrink lag near the end to cut the drain tail
N_WARMUP = 4       # junk matmuls to ramp the PE p-state during DMA wait

_prog_cache = {}


class FillSched:
    """Named-unit fill scheduler. Units are atomic (they share the mm_ps
    pool and must not interleave with each other); steps within the
    active unit are paced out by PE cost (matmul steps cost 1, DVE/DMA
    steps cost 0). require(name) forces full emission of every unit up
    to and including `name` — emission order defines Tile dependencies,
    so any unit a consumer reads from MUST be emitted (not just queued)
    before the consumer."""

    def __init__(self):
        self.order = deque()      # (name, deque((fn, cost)))
        self.cur_name = None
        self.cur = deque()
        self.done = set()

    def add(self, name, unit):
        self.order.append((name, deque(unit)))

    def add_front(self, name, unit):
        self.order.appendleft((name, deque(unit)))

    def _finish_cur(self):
        while self.cur:
            self.cur.popleft()[0]()
        if self.cur_name is not None:
            self.done.add(self.cur_name)
            self.cur_name = None

    def pop_steps(self, budget):
        while budget > 0:
            if not self.cur:
                if self.cur_name is not None:
                    self.done.add(self.cur_name)
                    self.cur_name = None
                if not self.order:
                    return
                self.cur_name, self.cur = self.order.popleft()
            fn, cost = self.cur.popleft()
            fn()
            budget -= cost
        if not self.cur and self.cur_name is not None:
            self.done.add(self.cur_name)
            self.cur_name = None

    def require(self, name):
        if name in self.done:
            return
        if self.cur_name is not None:
            if self.cur_name == name:
                self._finish_cur()
                return
            self._finish_cur()
        while name not in self.done:
            assert self.order, f"unit {name} was never queued"
            self.cur_name, self.cur = self.order.popleft()
            self._finish_cur()

    def drain(self):
        self._finish_cur()
        while self.order:
            self.cur_name, self.cur = self.order.popleft()
            self._finish_cur()


def build_program():
    nc = bacc.Bacc("TRN2", target_bir_lowering=False, debug=False)

    xq = nc.dram_tensor("xq", [8, 128, 8, 512], BF16, kind="ExternalInput")
    xk = nc.dram_tensor("xk", [8, 128, 8, 512], BF16, kind="ExternalInput")
    xv = nc.dram_tensor("xv", [8, 128, 8, 512], BF16, kind="ExternalInput")
    # pre-tiled on host: w[p, c, d] = W[c*128+p, d] -> contiguous DMA
    wq = nc.dram_tensor("wq", [128, 8, 128], BF16, kind="ExternalInput")
    wk = nc.dram_tensor("wk", [128, 8, 128], BF16, kind="ExternalInput")
    wv = nc.dram_tensor("wv", [128, 8, 128], BF16, kind="ExternalInput")
    wo = nc.dram_tensor("wo", [128, DM], BF16, kind="ExternalInput")
    bqk = nc.dram_tensor("bqk", [128, 2], F32, kind="ExternalInput")
    bv = nc.dram_tensor("bv", [128, 1], F32, kind="ExternalInput")
    outT = nc.dram_tensor("outT", [8, 128, 8, 512], BF16, kind="ExternalOutput")

    Exp = mybir.ActivationFunctionType.Exp

    with tile.TileContext(nc) as tc:
        with (
            tc.tile_pool(name="const", bufs=1) as const,
            tc.tile_pool(name="big", bufs=1) as big,
            tc.tile_pool(name="xpool", bufs=1) as xpool,
            tc.tile_pool(name="attn", bufs=AV_LAG + 2) as attnp,
            tc.tile_pool(name="norm", bufs=2) as normp,
            tc.tile_pool(name="osb", bufs=2) as osb,
            tc.tile_pool(name="vtpool", bufs=3) as vtpool,
            tc.tile_pool(name="mm_ps", bufs=2, space="PSUM") as mm_ps,
            tc.tile_pool(name="sc_ps", bufs=2, space="PSUM") as sc_ps,
            tc.tile_pool(name="y_ps", bufs=2, space="PSUM") as y_ps,
        ):
            # ---- weights / biases.  Issue order per engine == execution
            # order; sync and scalar each issue a DMA in ~0.6us, transfers
            # are FIFO per queue with ~5us first-transfer latency, and the
            # DGE queue blocks after ~4 outstanding.  So: tiny bqk first
            # (it gates the first projection eviction), 2 big chunks per
            # x tile, k-side on sync / q-side on scalar. ----
            bqk_sb = const.tile([128, 2], F32, tag="bqk")
            nc.scalar.dma_start(bqk_sb[:], bqk[:, :])
            wk_sb = const.tile([128, 8, 128], BF16, tag="wk")
            nc.sync.dma_start(wk_sb[:], wk[:])
            wq_sb = const.tile([128, 8, 128], BF16, tag="wq")
            nc.scalar.dma_start(wq_sb[:], wq[:])

            # ---- persistent activations ----
            qT = big.tile([128, TOK], BF16, tag="qT")     # [hd, tok]
            kT = big.tile([128, TOK], BF16, tag="kT")     # [hd, tok]
            # v blocks: 32 token-blocks of [128 tok, 2*(64+1)]; col 64 of each
            # per-head group is the ones column (softmax denominator trick)
            vA = big.tile([128, 32 * 130], BF16, tag="vA")
            yT = big.tile([128, TOK], BF16, tag="yT")     # attn out [hd, tok]

            nc.vector.memset(
                vA[:].rearrange("p (b h c) -> p b h c", h=2, c=65)[:, :, :, 64:65], 1.0
            )

            prefetched = {}

            # chunked first tiles: 4 chunks of 256KB so projections can
            # start as soon as the first chunk lands (the DMA subsystem
            # ramps slowly in the first ~15us — small chunks flow better)
            def prefetch_chunked(key, tb, eng):
                dram = {"q": xq, "k": xk, "v": xv}[key]
                t = xpool.tile([128, 8, 512], BF16,
                               tag=f"x{key}{tb % 4}", name="xt")
                for j in range(4):
                    eng.dma_start(t[:, 2 * j:2 * j + 2, :],
                                  dram[tb][:, 2 * j:2 * j + 2, :])
                prefetched[(key, tb)] = t

            prefetch_chunked("k", 0, nc.sync)
            prefetch_chunked("q", 0, nc.scalar)

            # [128,128] identity for the paired-head V transposes (built
            # on gpsimd - no DMA), and a junk warm-up operand (memset on
            # the idle DVE) so warm-ups need no DMA at all
            ident = const.tile([128, 128], BF16, tag="ident")
            make_identity(nc, ident[:])
            warm_sb = const.tile([128, 512], BF16, tag="warm")
            nc.vector.memset(warm_sb[:], 0.25)

            # ---- PE p-state warm-up: junk matmuls while the first x
            # chunks are in flight (never read; sc tiles are recycled).
            # A cold PE runs at 0.65-1.2GHz until ~3us of continuous work.
            def warm_mms(k):
                for w in range(k):
                    wps = sc_ps.tile([128, 1024], F32, tag="sc", name="warm")
                    nc.tensor.matmul(wps[:, 0:512], lhsT=ident[:],
                                     rhs=warm_sb[:], start=True, stop=True)
                    nc.tensor.matmul(wps[:, 512:1024], lhsT=ident[:],
                                     rhs=warm_sb[:], start=True, stop=True)

            warm_mms(N_WARMUP)

            def prefetch(key, tb):
                dram = {"q": xq, "k": xk, "v": xv}[key]
                # per-slot tag (bufs=1): batch-1's (key, tb+4) reuses exactly
                # the buffer of (key, tb), with a WAR dep on its reads
                t = xpool.tile([128, 8, 512], BF16,
                               tag=f"x{key}{tb % 4}", name="xt")
                nc.sync.dma_start(t[:], dram[tb])
                prefetched[(key, tb)] = t

            # remaining batch-0 prefetches + late weights, all on sync, in
            # deadline order (k1 @it4, k2 @it8, v0 @it12(AV), k3 @it12,
            # q1 @it16, wo @ first out-proj ~it30, ...)
            prefetch("k", 1)
            prefetch("k", 2)
            wv_sb = const.tile([128, 8, 128], BF16, tag="wv")
            nc.sync.dma_start(wv_sb[:], wv[:])
            prefetch("v", 0)
            bv_sb = const.tile([128, 1], F32, tag="bv")
            nc.sync.dma_start(bv_sb[:], bv[:, :])
            prefetch("k", 3)
            prefetch("v", 1)
            prefetch("q", 1)
            prefetch("v", 2)
            prefetch("q", 2)
            prefetch("v", 3)
            prefetch("q", 3)
            wo_sb = const.tile([128, DM], BF16, tag="wo")
            nc.sync.dma_start(wo_sb[:], wo[:, :])

            def proj_qk_steps(tb, which):
                """One qk projection as 8 single-MM closures (last one evicts)."""
                key, w_sb, dstT, bcol = (
                    ("q", wq_sb, qT, 0),
                    ("k", wk_sb, kT, 1),
                )[which]
                state = {}

                def step(mc):
                    if mc == 0:
                        state["xt"] = prefetched.pop((key, tb))
                        state["ps"] = mm_ps.tile([128, 512], F32, tag="mm", name="ps")
                    nc.tensor.matmul(
                        state["ps"][:], lhsT=w_sb[:, mc, :],
                        rhs=state["xt"][:, mc, :],
                        start=(mc == 0), stop=(mc == 7),
                    )
                    if mc == 7:
                        nc.vector.tensor_scalar_add(
                            dstT[:, tb * 512:(tb + 1) * 512], state["ps"][:],
                            bqk_sb[:, bcol:bcol + 1],
                        )
                        if tb < 4:
                            # batch-1 prefetch reuses this tile's buffer; it
                            # must be EMITTED after the last read of the old
                            # tile or the DMA races the projection
                            prefetch(key, tb + 4)
                return [(lambda mc=mc: step(mc), 1) for mc in range(8)]

            vstate = {}

            def proj_v_mm_steps(tb):
                """V projection matmuls: 8 single-MM closures + a bias
                evict to an SBUF staging tile (DVE)."""
                state = vstate.setdefault(tb, {})

                def step(mc):
                    if mc == 0:
                        state["xt"] = prefetched.pop(("v", tb))
                        state["ps"] = mm_ps.tile([128, 512], F32, tag="mm", name="ps")
                    nc.tensor.matmul(
                        state["ps"][:], lhsT=wv_sb[:, mc, :],
                        rhs=state["xt"][:, mc, :],
                        start=(mc == 0), stop=(mc == 7),
                    )
                    if mc == 7:
                        state["vt"] = vtpool.tile(
                            [128, 512], BF16, tag="vt", name="vt")
                        nc.vector.tensor_scalar_add(
                            state["vt"][:], state["ps"][:], bv_sb[:, 0:1])
                    if mc == 7 and tb < 4:
                        prefetch("v", tb + 4)

                return [(lambda mc=mc: step(mc), 1) for mc in range(8)]

            def proj_v_t_steps(tb):
                """V transposes: 4 paired-head [128,128] PE transposes + 2
                DVE scatter copies each into the 65-stride vA layout.
                A SEPARATE fill unit scheduled one unit after the matmuls,
                so the in-order PE doesn't reach the first transpose before
                the DVE bias-evict has written the vt staging tile."""
                state = vstate  # read via tb at call time

                def tstep(j):
                    base = (tb * 4 + j) * 130
                    vt = vstate[tb]["vt"]
                    # [128 hd, 128 tok] -> [128 tok, 128 hd] in one shot
                    tp = mm_ps.tile([128, 128], BF16, tag="mm", name="tp")
                    nc.tensor.transpose(
                        tp[:], vt[:, j * 128:(j + 1) * 128], ident[:])
                    for hl in range(2):
                        nc.vector.tensor_copy(
                            vA[:, base + hl * 65: base + hl * 65 + 64],
                            tp[:, hl * 64:(hl + 1) * 64])

                return [(lambda j=j: tstep(j), 1) for j in range(4)]

            def out_proj_steps(n, qtr, tail=False):
                """One 512-token output-projection stripe: 8 (MM + bf16 copy)
                closures; the last also DMAs the stripe out. In tail mode the
                exp stream is over, so the dead score PSUM banks give 4 mm
                slots, ScalarE helps with the casts, and the DMA is split
                into 4 chunks to overlap them."""
                tb = n * 4 + qtr
                state = {}

                def step(mb):
                    if mb == 0:
                        state["o"] = osb.tile(
                            [128, 8, 512], BF16, tag="o", name="o_sb")
                    if tail:
                        if mb % 2 == 0:
                            state["ps"] = sc_ps.tile(
                                [128, 1024], F32, tag="sc", name="ps")
                        ps = state["ps"][:, (mb % 2) * 512:(mb % 2) * 512 + 512]
                    else:
                        ps = mm_ps.tile([128, 512], F32, tag="mm", name="ps")[:]
                    nc.tensor.matmul(
                        ps, lhsT=wo_sb[:, mb * 128:(mb + 1) * 128],
                        rhs=yT[:, tb * 512:(tb + 1) * 512],
                        start=True, stop=True,
                    )
                    if tail and mb % 2 == 0:
                        nc.scalar.copy(state["o"][:, mb, :], ps)
                    else:
                        nc.vector.tensor_copy(state["o"][:, mb, :], ps)
                    if tail and mb % 2 == 1:
                        nc.sync.dma_start(
                            outT[tb, :, mb - 1:mb + 1, :],
                            state["o"][:, mb - 1:mb + 1, :])
                    elif not tail and mb == 7:
                        nc.sync.dma_start(outT[tb], state["o"][:])
                return [(lambda mb=mb: step(mb), 1) for mb in range(8)]

            fill = FillSched()
            pending_fronts = []

            def norm_qtr(n, qtr, yaccs):
                kq0 = n * SEQ + qtr * 512
                tail = (n == NB - 1 and qtr == 3)
                if not tail:
                    ycps = []
                    for h in range(2):
                        # both release copies FIRST: they free the yacc PSUM
                        # buffers the next quarter's first AVs are waiting on;
                        # the rest of the chain runs off the PE critical path.
                        # (gpsimd can't access PSUM; custom DVE ops can't
                        # either)
                        ycp = normp.tile([65, 512], F32, tag="ycp", name="ycp")
                        nc.vector.tensor_copy(ycp[:], yaccs[h][:])
                        ycps.append(ycp)
                else:
                    # final quarter: nothing follows — read yacc PSUM
                    # directly, skip the release copies, and pull the
                    # second denominator copy onto the idle ScalarE.
                    # (The tail runs under a ~50% hardware power cap, so
                    # keep-warm junk matmuls don't help — measured.)
                    ycps = yaccs
                dsbs = []
                for h in range(2):
                    # custom DVE op needs a partition-0 input
                    dsb = normp.tile([1, 512], F32, tag="dsb", name="dsb")
                    if tail and h == 1:
                        nc.scalar.copy(dsb[:], ycps[h][64:65, :])
                    else:
                        nc.vector.tensor_copy(dsb[:], ycps[h][64:65, :])
                    dsbs.append(dsb)
                for h in range(2):
                    hp = h * 64
                    ycp = ycps[h]
                    recip = normp.tile([1, 512], F32, tag="recip", name="recip")
                    nc.vector.reciprocal_approx_fast(recip[:], dsbs[h])
                    bcast = normp.tile([64, 512], F32, tag="bcast", name="bcast")
                    nc.gpsimd.partition_broadcast(bcast[:], recip[:])
                    nc.vector.tensor_mul(
                        yT[hp:hp + 64, kq0:kq0 + 512],
                        ycp[0:64, :], bcast[:],
                    )
                if tail:
                    fill.add_front(f"out{n}{qtr}", out_proj_steps(n, qtr, tail))
                else:
                    # defer one iteration: the in-order PE must not reach
                    # the stripe's first matmul before the ~4us DVE/gpsimd
                    # norm chain has written yT (measured ~1.3us stalls at
                    # every quarter boundary with immediate add_front)
                    pending_fronts.append(
                        (f"out{n}{qtr}", out_proj_steps(n, qtr, tail)))

            def do_av(at, n, qtr, lb, yaccs):
                lt = n * 16 + lb
                fill.require(f"v{n * 4 + lb // 4}")
                for h in range(2):
                    nc.tensor.matmul(
                        yaccs[h][:],
                        lhsT=vA[:, lt * 130 + h * 65: lt * 130 + h * 65 + 65],
                        rhs=at[:, h * 512:(h + 1) * 512],
                        start=(lb == 0), stop=(lb == 15),
                    )
                if lb == 15:
                    norm_qtr(n, qtr, yaccs)

            # ---- upfront: k0 + q0 projections only ----
            for f, _ in proj_qk_steps(0, 1):
                f()
            for f, _ in proj_qk_steps(0, 0):
                f()
            fill.done.update({"k0", "q0"})

            # deadline order: scores(lb) need k-tb(lb//4) / q-tb(qtr);
            # AV (lag 12) needs v-tb((lb-12)//4); batch 1 follows
            # (batch-1 prefetches are emitted by the consumption hooks
            # inside proj_*_steps — buffer-exact, race-free)
            units = [
                ("k1", proj_qk_steps(1, 1)), ("k2", proj_qk_steps(2, 1)),
                ("v0m", proj_v_mm_steps(0)), ("k3", proj_qk_steps(3, 1)),
                ("v0", proj_v_t_steps(0)),
                ("v1m", proj_v_mm_steps(1)), ("q1", proj_qk_steps(1, 0)),
                ("v1", proj_v_t_steps(1)),
                ("v2m", proj_v_mm_steps(2)), ("q2", proj_qk_steps(2, 0)),
                ("v2", proj_v_t_steps(2)),
                ("v3m", proj_v_mm_steps(3)), ("q3", proj_qk_steps(3, 0)),
                ("v3", proj_v_t_steps(3)),
            ]
            units += [
                ("k4", proj_qk_steps(4, 1)), ("q4", proj_qk_steps(4, 0)),
                ("k5", proj_qk_steps(5, 1)), ("k6", proj_qk_steps(6, 1)),
                ("v4m", proj_v_mm_steps(4)), ("k7", proj_qk_steps(7, 1)),
                ("v4", proj_v_t_steps(4)),
                ("v5m", proj_v_mm_steps(5)), ("q5", proj_qk_steps(5, 0)),
                ("v5", proj_v_t_steps(5)),
                ("v6m", proj_v_mm_steps(6)), ("q6", proj_qk_steps(6, 0)),
                ("v6", proj_v_t_steps(6)),
                ("v7m", proj_v_mm_steps(7)), ("q7", proj_qk_steps(7, 0)),
                ("v7", proj_v_t_steps(7)),
            ]
            for name, unit in units:
                fill.add(name, unit)

            # ---- the continuous attention stream ----
            pend = deque()
            qtr_yaccs = {}
            NIT = NB * 64
            for i in range(NIT):
                while pending_fronts:
                    fill.add_front(*pending_fronts.pop())
                n, r = divmod(i, 64)
                qtr, lb = divmod(r, 16)
                lt = n * 16 + lb
                kq0 = n * SEQ + qtr * 512
                if lb == 0:
                    qtr_yaccs[(n, qtr)] = [
                        y_ps.tile([65, 512], F32, tag="yacc", name="yacc")
                        for _ in range(2)
                    ]
                fill.require(f"k{n * 4 + lb // 4}")
                fill.require(f"q{n * 4 + qtr}")
                # 4/3/2/1: stretch the fill queue into the late stream —
                # after ~i=95 the old 4/3/2 pacing left the queue empty
                # while scores+AV (~780ns) under-fill the ~1.1us exp
                # cadence, so the PE idled exactly where the late
                # quarters' norm chains also stall
                budget = 4 if i < 24 else (3 if i < 48 else (2 if i < 80 else 1))
                # interleave fill steps between the stream's matmuls so
                # their PSUM-eviction WARs are satisfied by arrival
                fill.pop_steps(1)
                budget -= 1
                sp = sc_ps.tile([128, 1024], F32, tag="sc", name="sp")
                for h in range(2):
                    # K=64, base partitions 0/64 -> concurrent row-tiles
                    nc.tensor.matmul(
                        sp[:, h * 512:(h + 1) * 512],
                        lhsT=kT[h * 64:(h + 1) * 64, lt * 128:(lt + 1) * 128],
                        rhs=qT[h * 64:(h + 1) * 64, kq0:kq0 + 512],
                        start=True, stop=True,
                    )
                at = attnp.tile([128, 1024], BF16, tag="at", name="at")
                nc.scalar.activation(at[:], sp[:], Exp, scale=0.125)
                pend.append((at, n, qtr, lb, qtr_yaccs[(n, qtr)]))
                fill.pop_steps(1)
                budget -= 1
                limit = AV_LAG if i < NIT - (AV_LAG - AV_LAG_TAIL) else AV_LAG_TAIL
                # drain AVs two l-blocks at a time (every other iteration):
                # grouping full-array AV matmuls halves the PE row-config
                # switches against the half-array score pairs
                if i % 2 == 1 or len(pend) > limit + 1:
                    while len(pend) > limit - 1 and pend:
                        args = pend.popleft()
                        do_av(*args)
                        if args[3] == 15:
                            del qtr_yaccs[(args[1], args[2])]
                if budget:
                    fill.pop_steps(budget)
            for args in pend:
                do_av(*args)
            while pending_fronts:
                fill.add_front(*pending_fronts.pop())
            fill.drain()

    nc.compile()
    return nc


def get_program():
    if "nc" not in _prog_cache:
        _prog_cache["nc"] = build_program()
    return _prog_cache["nc"]


def _tile_x(x):
    # [TOK, DM] f32 -> bf16 tiles [8 tb, 128 p, 8 mc, 512 f]:
    # t[tb,p,mc,f] = x[tb*512+f, mc*128+p]
    t = x.reshape(8, 512, 8, 128).astype(BFNP)   # [tb, f, mc, p]
    return np.ascontiguousarray(np.transpose(t, (0, 3, 2, 1)))


def make_in_maps(query, key, value, Wq, bq, Wk, bk, Wv, bv, Wo):
    """Host-side shard + layout. Returns list of 8 per-core input dicts."""
    xq = _tile_x(query.reshape(TOK, DM))
    xk = _tile_x(key.reshape(TOK, DM))
    xv = _tile_x(value.reshape(TOK, DM))

    def _tile_w(W, h0):
        # W[h,d,m] slice -> [m, hl*64+d] -> pre-tiled [p, mc, hd]
        wf = np.transpose(W[h0:h0 + HC], (2, 0, 1)).reshape(DM, 128)
        return np.ascontiguousarray(
            wf.reshape(8, 128, 128).transpose(1, 0, 2)).astype(BFNP)

    in_maps = []
    for c in range(N_CORES):
        h0 = HC * c
        wq_c = _tile_w(Wq, h0)
        wk_c = _tile_w(Wk, h0)
        wv_c = _tile_w(Wv, h0)
        wo_c = np.ascontiguousarray(
            Wo[:, 128 * c:128 * (c + 1)].T).astype(BFNP)
        bqk_c = np.stack(
            [bq[h0:h0 + HC].reshape(128), bk[h0:h0 + HC].reshape(128)], axis=1
        ).astype(np.float32)
        bv_c = bv[h0:h0 + HC].reshape(128, 1).astype(np.float32)
        in_maps.append({
            "xq": xq, "xk": xk, "xv": xv,
            "wq": wq_c, "wk": wk_c, "wv": wv_c, "wo": wo_c,
            "bqk": bqk_c, "bv": bv_c,
        })
    return in_maps


def untile_out(res_list):
    """Sum per-core tiled bf16 partials in f32 -> [DM, TOK] f32."""
    acc = np.zeros((8, 128, 8, 512), np.float32)
    for r in res_list:
        acc += r["outT"].astype(np.float32)
    # [tb, p, mb, f] -> [mb*128+p, tb*512+f]
    return np.ascontiguousarray(np.transpose(acc, (2, 1, 0, 3))).reshape(DM, TOK)


def kernel(query, key, value, Wq, bq, Wk, bk, Wv, bv, Wo, bo):
    nc = get_program()
    in_maps = make_in_maps(query, key, value, Wq, bq, Wk, bk, Wv, bv, Wo)
    res = run_bass_kernel_spmd(nc, in_maps, list(range(N_CORES)))
    out_t = untile_out(res.results)
    out = out_t.T.reshape(NB, SEQ, DM) + bo.astype(np.float32)
    return out
